# revision 34
# baseline (speedup 1.0000x reference)
"""Ewald reciprocal-space sum on 8 Trainium2 NeuronCores.

Math: for each system b, S(k) = sum_a q_a e^{i k.r_a} over the static
integer k-grid n in [-10,10]^3, k = n @ G, G = 2*pi*inv(cell)^T. With
phases phi_d = (r @ inv(cell))_d (turns), k.r = 2*pi*(n1*phi1 + n2*phi2
+ n3*phi3). Two prunings shrink the device work:
  - conjugate symmetry in the (n2,n3) plane: only n2 >= 0 is evaluated;
    the n2 < 0 half is recovered on the host from the same partial sums;
  - the reference's k_sq <= (2*pi/dl)^2 mask zeroes every mode with
    |n|^2 > 100, so the rectangle only needs n1,n2 in 0..9, n3 in -9..9;
    the three surviving axis modes (10,0,0),(0,10,0),(0,0,10) are added
    exactly on the host (three dot products).

Host precomputes (f64, cheap O(atoms) work):
  F2[a,j] = frac-centered(j*phi2), j in 0..9
  F3[a,j] = frac-centered(j*phi3), j in -9..9, and F3c = F3 - 1/4
  lhs[a]  = [q*cos(2*pi*j*phi1) | q*sin(...)] for j in 0..9 (20 cols)
Device work per core (SPMD, core c owns half the atoms of system c//2),
per 128-atom chunk t (8 chunks):
  V[b,j3,j2] = wrap(F2[j2] + F3ext[b,j3]) in [-1/2,1/2]  (1 fused DVE op)
  AA = Sin(-2*pi*V) -> [-sin(th23) | cos(th23)]          (ACT, fused x2)
  ps += lhs_t^T @ AA   [20 x 380] PSUM-accumulated       (1 PE matmul)
Host combines the 4 quadrant blocks of ps into S over all surviving
pairs via the mirror identity and applies the reference's k-space
weights.
"""

import numpy as np

# ---- problem constants (hardcoded per contract) ----
B = 4
N_PER = 2000
NK = 10                      # full k-grid extent: n in [-NK, NK]
NH1 = 10                     # n1 values 0..9 on device
NH2 = 10                     # n2 values 0..9 on device
NJ3 = 19                     # n3 values -9..9 on device
NRECT = NH2 * NJ3            # 190 pairs in the pruned rectangle
OUT_COLS = 2 * NRECT         # 380
CC = NH2 + 2 * NJ3           # 48 ftab cols per chunk: F2 | F3 | F3c
DL = 2.0
SIGMA = 1.0
EPS = 1e-6
NORM = 90.0474
TWOPI = 2.0 * np.pi

N_CORES = 8
CORES_PER_SYS = 2
ATOMS_PER_CORE = (B * N_PER) // N_CORES     # 1000
CHUNKS = 8                                  # ceil(1000/128)
PADN = CHUNKS * 128                         # 1024

FTAB_COLS = CHUNKS * CC                     # 384
LHS_COLS = CHUNKS * 2 * NH1                 # 160

_CACHE = {}


def _build_nc():
    import concourse.bacc as bacc
    import concourse.mybir as mybir
    import concourse.tile as tile

    # cheaper TileContext exit: the Bass preamble re-clears the whole
    # kernel sem range at every execution, so the exit-time sem clear and
    # second all-engine barrier are redundant for this single-context
    # kernel; keep drain + one barrier.
    def _cheap_drain_and_barrier(self, tick_clock, wait_clock):
        drain_inst = self.nc.sync.drain()
        wait_clock.add_sem_waits(
            drain_inst.ins, tile.ScopedClock({None: tick_clock.global_clock})
        )
        popped = self.nc._tile_sem_poison_stack.pop()
        assert popped is self._sem_poison

    f32 = mybir.dt.float32
    bf16 = mybir.dt.bfloat16
    Act = mybir.ActivationFunctionType

    # fused custom DVE op: out = wrap(in0 + in1 + s0) into [-s1, s1] with
    # period 1 (turn space)
    import concourse.dve_ops as dve_ops

    if not hasattr(dve_ops, "ADD_WRAP_EWALD"):
        from concourse.dve_spec import C0, C1, Spec, Src0, Src1, lower
        from concourse.dve_uop import DveOpSpec

        _y = (Src0 + Src1) + C0

        def _ref(in0, in1, s0, s1, imm2):
            y = in0 + in1 + s0
            return y + (
                (y < -s1).astype(np.float32) - (y > s1).astype(np.float32)
            )

        _spec = Spec(body=_y + ((_y < -C1) - (_y > C1)), reference=_ref)
        _shas = {
            ver: DveOpSpec(
                name="ADD_WRAP_EWALD", opcode=0,
                uops=lower(_spec, ver=ver), rd1_en=True,
            ).sha(ver)
            for ver in ("v3", "v4")
        }
        _op = dve_ops.DveOp("ADD_WRAP_EWALD", _spec, subdim=False, uops_sha=_shas)
        dve_ops.OPS.append(_op)
        dve_ops._SUB_OPCODE_FOR_NAME[_op.name] = (
            dve_ops._CUSTOM_DVE_ROW_BASE + len(dve_ops.OPS) - 1
        )
        dve_ops.CUSTOM_DVE_SPECS[_op.name] = _spec
        dve_ops.ADD_WRAP_EWALD = _op
    AW = dve_ops.ADD_WRAP_EWALD

    tile.TileContext._drain_and_barrier = _cheap_drain_and_barrier
    nc = bacc.Bacc(None, target_bir_lowering=False)

    ftab = nc.dram_tensor("ftab", [128, FTAB_COLS], f32, kind="ExternalInput")
    lhs = nc.dram_tensor("lhs", [128, LHS_COLS], bf16, kind="ExternalInput")
    sout = nc.dram_tensor("sout", [2 * NH1, OUT_COLS], bf16, kind="ExternalOutput")

    with tile.TileContext(nc) as tc:
        with (
            tc.tile_pool(name="const", bufs=1) as cp,
            tc.tile_pool(name="work", bufs=4) as wp,
            tc.tile_pool(name="psum", bufs=1, space="PSUM") as pp,
        ):
            ft = cp.tile([128, FTAB_COLS], f32)
            # split so the first chunks land (and unblock the DVE) early;
            # all pieces go on sync's HWDGE ring: same-ring pieces still
            # pipeline, and a scalar-ring DMA would force a second
            # ACT_TABLE_LOAD (~1.3us) into the scalar program.
            for lo, hi in ((0, 3), (3, 8)):
                nc.sync.dma_start(
                    out=ft[:, lo * CC : hi * CC], in_=ftab[:, lo * CC : hi * CC]
                )
            lt = cp.tile([128, LHS_COLS], bf16)
            nc.gpsimd.dma_start(out=lt[:], in_=lhs[:])

            ps = pp.tile([2 * NH1, OUT_COLS], f32)

            def pair_args(t):
                # pair col = bj*10 + j2, bj = b*19 + j3 (b=0 sin, b=1 cos)
                f2 = ft[:, CC * t : CC * t + NH2]
                f3x = ft[:, CC * t + NH2 : CC * (t + 1)]
                return (
                    f2.unsqueeze(1).broadcast_to([128, 2 * NJ3, NH2]),
                    f3x.unsqueeze(2).broadcast_to([128, 2 * NJ3, NH2]),
                )

            # groups of chunks sharing one ACT op: pairs up front for
            # fewer/larger Sin calls, singles at the end so the pipeline
            # tail drains chunk-at-a-time.
            groups = [(0, 1), (2, 3, 4), (5, 6), (7,)]
            for grp in groups:
                V = wp.tile([128, len(grp) * OUT_COLS], f32)
                for h, t in enumerate(grp):
                    in0, in1 = pair_args(t)
                    nc.vector._custom_dve(
                        AW,
                        out=V[:, OUT_COLS * h : OUT_COLS * (h + 1)].rearrange(
                            "p (bj j2) -> p bj j2", j2=NH2
                        ),
                        in0=in0, in1=in1, s0=0.0, s1=0.5,
                    )
                AA = wp.tile([128, len(grp) * OUT_COLS], bf16)
                nc.scalar.activation(
                    out=AA[:], in_=V[:], func=Act.Sin, bias=0.0, scale=-TWOPI
                )
                for h, t in enumerate(grp):
                    nc.tensor.matmul(
                        out=ps[:],
                        lhsT=lt[:, 2 * NH1 * t : 2 * NH1 * (t + 1)],
                        rhs=AA[:, OUT_COLS * h : OUT_COLS * (h + 1)],
                        start=(t == 0), stop=(t == CHUNKS - 1),
                    )

            so = wp.tile([2 * NH1, OUT_COLS], bf16)
            nc.vector.tensor_copy(out=so[:], in_=ps[:])
            nc.sync.dma_start(out=sout[:], in_=so[:])

    nc.compile()
    return nc


def _get_nc():
    if "nc" not in _CACHE:
        _CACHE["nc"] = _build_nc()
    return _CACHE["nc"]


def _bf16(x):
    import ml_dtypes
    return x.astype(ml_dtypes.bfloat16)


def _chunk_major(x, w):
    """atom a = t*128 + p  ->  [p, t*w + j]"""
    return x.reshape(CHUNKS, 128, w).transpose(1, 0, 2).reshape(128, CHUNKS * w)


def _phi_q(q, r, cell, c):
    b = c // CORES_PER_SYS
    half = c % CORES_PER_SYS
    lo = b * N_PER + half * ATOMS_PER_CORE
    rs = r[lo : lo + ATOMS_PER_CORE].astype(np.float64)
    qs = q[lo : lo + ATOMS_PER_CORE, 0].astype(np.float64)
    minv = np.linalg.inv(cell[b].astype(np.float64))
    return (rs @ minv) % 1.0, qs


def _host_inputs(q, r, cell):
    j1 = np.arange(0, NH1, dtype=np.float64)
    j2 = np.arange(0, NH2, dtype=np.float64)
    j3 = np.arange(-(NJ3 // 2), NJ3 // 2 + 1, dtype=np.float64)
    maps = []
    for c in range(N_CORES):
        phi, qs = _phi_q(q, r, cell, c)
        phi_p = np.zeros((PADN, 3)); phi_p[:ATOMS_PER_CORE] = phi
        q_p = np.zeros(PADN); q_p[:ATOMS_PER_CORE] = qs
        ang1 = TWOPI * np.outer(phi_p[:, 0], j1)
        lhs = np.concatenate(
            [np.cos(ang1) * q_p[:, None], np.sin(ang1) * q_p[:, None]], axis=1
        )
        t2 = np.outer(phi_p[:, 1], j2); F2 = t2 - np.round(t2)
        t3 = np.outer(phi_p[:, 2], j3); F3 = t3 - np.round(t3)
        # per-chunk groups of 48 cols: F2 (10) | F3 (19) | F3 - 1/4 (19)
        ftab = _chunk_major(
            np.concatenate([F2, F3, F3 - 0.25], axis=1), CC
        ).astype(np.float32)
        maps.append(
            {"ftab": ftab, "lhs": _bf16(_chunk_major(lhs, 2 * NH1))}
        )
    return maps


def _host_weights(cell):
    """w[b, n1(0..10), n2(-10..10), n3(-10..10)] mirroring the reference."""
    k_sq_max = (TWOPI / DL) ** 2
    sigma_sq_half = SIGMA ** 2 / 2.0
    rng = np.arange(-NK, NK + 1, dtype=np.float64)
    n1, n2, n3 = np.meshgrid(rng[NK:], rng, rng, indexing="ij")
    nvec = np.stack([n1.ravel(), n2.ravel(), n3.ravel()], axis=1)
    hemi = (
        (nvec[:, 0] > 0)
        | ((nvec[:, 0] == 0) & (nvec[:, 1] > 0))
        | ((nvec[:, 0] == 0) & (nvec[:, 1] == 0) & (nvec[:, 2] > 0))
    )
    ws = []
    for b in range(B):
        cb = cell[b].astype(np.float64)
        G = TWOPI * np.linalg.inv(cb).T
        kvec = nvec @ G
        k_sq = np.sum(kvec ** 2, axis=1)
        mask = (k_sq > 0) & (k_sq <= k_sq_max) & hemi
        kfac = np.exp(-sigma_sq_half * k_sq) / (k_sq + EPS)
        vol = np.linalg.det(cb)
        ws.append(np.where(mask, 2.0 * kfac, 0.0) / vol)
    return np.stack(ws).reshape(B, NK + 1, 2 * NK + 1, 2 * NK + 1)


def kernel(q, r, cell, batch):
    from concourse.bass_utils import run_bass_kernel_spmd

    q = np.asarray(q)
    r = np.asarray(r)
    cell = np.asarray(cell)

    nc = _get_nc()
    in_maps = _host_inputs(q, r, cell)
    res = run_bass_kernel_spmd(nc, in_maps, core_ids=list(range(N_CORES))).results

    w = _host_weights(cell)
    pot = np.zeros(B, np.float64)
    for b in range(B):
        M = (
            res[b * CORES_PER_SYS]["sout"].astype(np.float64)
            + res[b * CORES_PER_SYS + 1]["sout"].astype(np.float64)
        )
        # pair blocks are [n3, n2]-ordered: col = j3*10 + n2
        Crs = -M[0:NH1, 0:NRECT].reshape(NH1, NJ3, NH2)    # sum q c1 sin(th23)
        Css = -M[NH1 : 2 * NH1, 0:NRECT].reshape(NH1, NJ3, NH2)
        Crc = M[0:NH1, NRECT:OUT_COLS].reshape(NH1, NJ3, NH2)
        Csc = M[NH1 : 2 * NH1, NRECT:OUT_COLS].reshape(NH1, NJ3, NH2)
        wb = w[b]
        # rect (n1 0..9, n2 0..9, n3 -9..9) -> full idx (n1, 10+n2, 10+n3)
        w_dir = wb[0:NH1, NK : NK + NH2, 1 : 2 * NK].transpose(0, 2, 1)
        w_mir = wb[0:NH1, NK:0:-1, 2 * NK - 1 : 0 : -1].copy()  # (-n2, -n3)
        w_mir[:, 0, :] = 0.0                      # n2=0 row counted once
        w_mir = w_mir.transpose(0, 2, 1)
        recip = (
            np.sum(w_dir * ((Crc - Css) ** 2 + (Crs + Csc) ** 2))
            + np.sum(w_mir * ((Crc + Css) ** 2 + (Csc - Crs) ** 2))
        )
        # the three surviving |n|=10 axis modes, exact on host
        lo = b * N_PER
        qs = q[lo : lo + N_PER, 0].astype(np.float64)
        minv = np.linalg.inv(cell[b].astype(np.float64))
        phi = (r[lo : lo + N_PER].astype(np.float64) @ minv) % 1.0
        for d, widx in ((0, (NK, NK, NK)), (1, (0, 2 * NK, NK)), (2, (0, NK, 2 * NK))):
            ang = TWOPI * NK * phi[:, d]
            sr = np.sum(qs * np.cos(ang)); si = np.sum(qs * np.sin(ang))
            recip += wb[widx] * (sr ** 2 + si ** 2)
        self_e = np.sum(qs ** 2) / (SIGMA * TWOPI ** 1.5)
        pot[b] = (recip - self_e) * NORM
    return pot.astype(np.float32)


# revision 35
# speedup vs baseline: 1.0307x; 1.0307x over previous
"""Ewald reciprocal-space sum on 8 Trainium2 NeuronCores.

Math: for each system b, S(k) = sum_a q_a e^{i k.r_a} over the static
integer k-grid n in [-10,10]^3, k = n @ G, G = 2*pi*inv(cell)^T. With
phases phi_d = (r @ inv(cell))_d (turns), k.r = 2*pi*(n1*phi1 + n2*phi2
+ n3*phi3). Two prunings shrink the device work:
  - conjugate symmetry in the (n2,n3) plane: only n2 >= 0 is evaluated;
    the n2 < 0 half is recovered on the host from the same partial sums;
  - the reference's k_sq <= (2*pi/dl)^2 mask zeroes every mode with
    |n|^2 > 100, so the rectangle only needs n1,n2 in 0..9, n3 in -9..9;
    the three surviving axis modes (10,0,0),(0,10,0),(0,0,10) are added
    exactly on the host (three dot products).

Host precomputes (f64, cheap O(atoms) work):
  F2[a,j] = frac-centered(j*phi2), j in 0..9
  F3[a,j] = frac-centered(j*phi3), j in -9..9, and F3c = F3 - 1/4
  lhs[a]  = [q*cos(2*pi*j*phi1) | q*sin(...)] for j in 0..9 (20 cols)
Device work per core (SPMD, core c owns half the atoms of system c//2),
per 128-atom chunk t (8 chunks):
  V[b,j3,j2] = wrap(F2[j2] + F3ext[b,j3]) in [-1/2,1/2]  (1 fused DVE op)
  AA = Sin(-2*pi*V) -> [-sin(th23) | cos(th23)]          (ACT, fused x2)
  ps += lhs_t^T @ AA   [20 x 380] PSUM-accumulated       (1 PE matmul)
Host combines the 4 quadrant blocks of ps into S over all surviving
pairs via the mirror identity and applies the reference's k-space
weights.
"""

import numpy as np

# ---- problem constants (hardcoded per contract) ----
B = 4
N_PER = 2000
NK = 10                      # full k-grid extent: n in [-NK, NK]
NH1 = 10                     # n1 values 0..9 on device
NH2 = 10                     # n2 values 0..9 on device
NJ3 = 19                     # n3 values -9..9 on device
NRECT = NH2 * NJ3            # 190 pairs in the pruned rectangle
OUT_COLS = 2 * NRECT         # 380
CC = NH2 + 2 * NJ3           # 48 ftab cols per chunk: F2 | F3 | F3c
DL = 2.0
SIGMA = 1.0
EPS = 1e-6
NORM = 90.0474
TWOPI = 2.0 * np.pi

N_CORES = 8
CORES_PER_SYS = 2
ATOMS_PER_CORE = (B * N_PER) // N_CORES     # 1000
CHUNKS = 8                                  # ceil(1000/128)
PADN = CHUNKS * 128                         # 1024

FTAB_COLS = CHUNKS * CC                     # 384
LHS_COLS = CHUNKS * 2 * NH1                 # 160

_CACHE = {}


def _build_nc():
    import concourse.bacc as bacc
    import concourse.mybir as mybir
    import concourse.tile as tile

    # cheaper TileContext exit: the Bass preamble re-clears the whole
    # kernel sem range at every execution, so the exit-time sem clear and
    # second all-engine barrier are redundant for this single-context
    # kernel; keep drain + one barrier.
    def _cheap_drain_and_barrier(self, tick_clock, wait_clock):
        drain_inst = self.nc.sync.drain()
        wait_clock.add_sem_waits(
            drain_inst.ins, tile.ScopedClock({None: tick_clock.global_clock})
        )
        popped = self.nc._tile_sem_poison_stack.pop()
        assert popped is self._sem_poison

    f32 = mybir.dt.float32
    bf16 = mybir.dt.bfloat16
    Act = mybir.ActivationFunctionType

    # fused custom DVE op: out = wrap(in0 + in1 + s0) into [-s1, s1] with
    # period 1 (turn space)
    import concourse.dve_ops as dve_ops

    if not hasattr(dve_ops, "ADD_WRAP_EWALD"):
        from concourse.dve_spec import C0, C1, Spec, Src0, Src1, lower
        from concourse.dve_uop import DveOpSpec

        _y = (Src0 + Src1) + C0

        def _ref(in0, in1, s0, s1, imm2):
            y = in0 + in1 + s0
            return y + (
                (y < -s1).astype(np.float32) - (y > s1).astype(np.float32)
            )

        _spec = Spec(body=_y + ((_y < -C1) - (_y > C1)), reference=_ref)
        _shas = {
            ver: DveOpSpec(
                name="ADD_WRAP_EWALD", opcode=0,
                uops=lower(_spec, ver=ver), rd1_en=True,
            ).sha(ver)
            for ver in ("v3", "v4")
        }
        _op = dve_ops.DveOp("ADD_WRAP_EWALD", _spec, subdim=False, uops_sha=_shas)
        dve_ops.OPS.append(_op)
        dve_ops._SUB_OPCODE_FOR_NAME[_op.name] = (
            dve_ops._CUSTOM_DVE_ROW_BASE + len(dve_ops.OPS) - 1
        )
        dve_ops.CUSTOM_DVE_SPECS[_op.name] = _spec
        dve_ops.ADD_WRAP_EWALD = _op
    AW = dve_ops.ADD_WRAP_EWALD

    tile.TileContext._drain_and_barrier = _cheap_drain_and_barrier
    nc = bacc.Bacc(None, target_bir_lowering=False)

    ftab = nc.dram_tensor("ftab", [128, FTAB_COLS], f32, kind="ExternalInput")
    lhs = nc.dram_tensor("lhs", [128, LHS_COLS], bf16, kind="ExternalInput")
    sout = nc.dram_tensor("sout", [2 * NH1, OUT_COLS], bf16, kind="ExternalOutput")

    with tile.TileContext(nc) as tc:
        with (
            tc.tile_pool(name="const", bufs=1) as cp,
            tc.tile_pool(name="work", bufs=5) as wp,
            tc.tile_pool(name="psum", bufs=1, space="PSUM") as pp,
        ):
            ft = cp.tile([128, FTAB_COLS], f32)
            # split so the first chunks land (and unblock the DVE) early;
            # all pieces go on sync's HWDGE ring: same-ring pieces still
            # pipeline, and a scalar-ring DMA would force a second
            # ACT_TABLE_LOAD (~1.3us) into the scalar program.
            for lo, hi in ((0, 3), (3, 8)):
                nc.sync.dma_start(
                    out=ft[:, lo * CC : hi * CC], in_=ftab[:, lo * CC : hi * CC]
                )
            lt = cp.tile([128, LHS_COLS], bf16)
            nc.gpsimd.dma_start(out=lt[:], in_=lhs[:])

            ps = pp.tile([2 * NH1, OUT_COLS], f32)

            def pair_args(t):
                # pair col = bj*10 + j2, bj = b*19 + j3 (b=0 sin, b=1 cos)
                f2 = ft[:, CC * t : CC * t + NH2]
                f3x = ft[:, CC * t + NH2 : CC * (t + 1)]
                return (
                    f2.unsqueeze(1).broadcast_to([128, 2 * NJ3, NH2]),
                    f3x.unsqueeze(2).broadcast_to([128, 2 * NJ3, NH2]),
                )

            # groups of chunks sharing one ACT op: pairs up front for
            # fewer/larger Sin calls, singles at the end so the pipeline
            # tail drains chunk-at-a-time.
            groups = [(0, 1), (2, 3, 4), (5, 6), (7,)]
            for grp in groups:
                V = wp.tile([128, len(grp) * OUT_COLS], f32)
                for h, t in enumerate(grp):
                    in0, in1 = pair_args(t)
                    nc.vector._custom_dve(
                        AW,
                        out=V[:, OUT_COLS * h : OUT_COLS * (h + 1)].rearrange(
                            "p (bj j2) -> p bj j2", j2=NH2
                        ),
                        in0=in0, in1=in1, s0=0.0, s1=0.5,
                    )
                AA = wp.tile([128, len(grp) * OUT_COLS], bf16)
                nc.scalar.activation(
                    out=AA[:], in_=V[:], func=Act.Sin, bias=0.0, scale=-TWOPI
                )
                for h, t in enumerate(grp):
                    nc.tensor.matmul(
                        out=ps[:],
                        lhsT=lt[:, 2 * NH1 * t : 2 * NH1 * (t + 1)],
                        rhs=AA[:, OUT_COLS * h : OUT_COLS * (h + 1)],
                        start=(t == 0), stop=(t == CHUNKS - 1),
                    )

            so = wp.tile([2 * NH1, OUT_COLS], bf16)
            nc.vector.tensor_copy(out=so[:], in_=ps[:])
            nc.sync.dma_start(out=sout[:], in_=so[:])

    nc.compile()
    return nc


def _get_nc():
    if "nc" not in _CACHE:
        _CACHE["nc"] = _build_nc()
    return _CACHE["nc"]


def _bf16(x):
    import ml_dtypes
    return x.astype(ml_dtypes.bfloat16)


def _chunk_major(x, w):
    """atom a = t*128 + p  ->  [p, t*w + j]"""
    return x.reshape(CHUNKS, 128, w).transpose(1, 0, 2).reshape(128, CHUNKS * w)


def _phi_q(q, r, cell, c):
    b = c // CORES_PER_SYS
    half = c % CORES_PER_SYS
    lo = b * N_PER + half * ATOMS_PER_CORE
    rs = r[lo : lo + ATOMS_PER_CORE].astype(np.float64)
    qs = q[lo : lo + ATOMS_PER_CORE, 0].astype(np.float64)
    minv = np.linalg.inv(cell[b].astype(np.float64))
    return (rs @ minv) % 1.0, qs


def _host_inputs(q, r, cell):
    j1 = np.arange(0, NH1, dtype=np.float64)
    j2 = np.arange(0, NH2, dtype=np.float64)
    j3 = np.arange(-(NJ3 // 2), NJ3 // 2 + 1, dtype=np.float64)
    maps = []
    for c in range(N_CORES):
        phi, qs = _phi_q(q, r, cell, c)
        phi_p = np.zeros((PADN, 3)); phi_p[:ATOMS_PER_CORE] = phi
        q_p = np.zeros(PADN); q_p[:ATOMS_PER_CORE] = qs
        ang1 = TWOPI * np.outer(phi_p[:, 0], j1)
        lhs = np.concatenate(
            [np.cos(ang1) * q_p[:, None], np.sin(ang1) * q_p[:, None]], axis=1
        )
        t2 = np.outer(phi_p[:, 1], j2); F2 = t2 - np.round(t2)
        t3 = np.outer(phi_p[:, 2], j3); F3 = t3 - np.round(t3)
        # per-chunk groups of 48 cols: F2 (10) | F3 (19) | F3 - 1/4 (19)
        ftab = _chunk_major(
            np.concatenate([F2, F3, F3 - 0.25], axis=1), CC
        ).astype(np.float32)
        maps.append(
            {"ftab": ftab, "lhs": _bf16(_chunk_major(lhs, 2 * NH1))}
        )
    return maps


def _host_weights(cell):
    """w[b, n1(0..10), n2(-10..10), n3(-10..10)] mirroring the reference."""
    k_sq_max = (TWOPI / DL) ** 2
    sigma_sq_half = SIGMA ** 2 / 2.0
    rng = np.arange(-NK, NK + 1, dtype=np.float64)
    n1, n2, n3 = np.meshgrid(rng[NK:], rng, rng, indexing="ij")
    nvec = np.stack([n1.ravel(), n2.ravel(), n3.ravel()], axis=1)
    hemi = (
        (nvec[:, 0] > 0)
        | ((nvec[:, 0] == 0) & (nvec[:, 1] > 0))
        | ((nvec[:, 0] == 0) & (nvec[:, 1] == 0) & (nvec[:, 2] > 0))
    )
    ws = []
    for b in range(B):
        cb = cell[b].astype(np.float64)
        G = TWOPI * np.linalg.inv(cb).T
        kvec = nvec @ G
        k_sq = np.sum(kvec ** 2, axis=1)
        mask = (k_sq > 0) & (k_sq <= k_sq_max) & hemi
        kfac = np.exp(-sigma_sq_half * k_sq) / (k_sq + EPS)
        vol = np.linalg.det(cb)
        ws.append(np.where(mask, 2.0 * kfac, 0.0) / vol)
    return np.stack(ws).reshape(B, NK + 1, 2 * NK + 1, 2 * NK + 1)


def kernel(q, r, cell, batch):
    from concourse.bass_utils import run_bass_kernel_spmd

    q = np.asarray(q)
    r = np.asarray(r)
    cell = np.asarray(cell)

    nc = _get_nc()
    in_maps = _host_inputs(q, r, cell)
    res = run_bass_kernel_spmd(nc, in_maps, core_ids=list(range(N_CORES))).results

    w = _host_weights(cell)
    pot = np.zeros(B, np.float64)
    for b in range(B):
        M = (
            res[b * CORES_PER_SYS]["sout"].astype(np.float64)
            + res[b * CORES_PER_SYS + 1]["sout"].astype(np.float64)
        )
        # pair blocks are [n3, n2]-ordered: col = j3*10 + n2
        Crs = -M[0:NH1, 0:NRECT].reshape(NH1, NJ3, NH2)    # sum q c1 sin(th23)
        Css = -M[NH1 : 2 * NH1, 0:NRECT].reshape(NH1, NJ3, NH2)
        Crc = M[0:NH1, NRECT:OUT_COLS].reshape(NH1, NJ3, NH2)
        Csc = M[NH1 : 2 * NH1, NRECT:OUT_COLS].reshape(NH1, NJ3, NH2)
        wb = w[b]
        # rect (n1 0..9, n2 0..9, n3 -9..9) -> full idx (n1, 10+n2, 10+n3)
        w_dir = wb[0:NH1, NK : NK + NH2, 1 : 2 * NK].transpose(0, 2, 1)
        w_mir = wb[0:NH1, NK:0:-1, 2 * NK - 1 : 0 : -1].copy()  # (-n2, -n3)
        w_mir[:, 0, :] = 0.0                      # n2=0 row counted once
        w_mir = w_mir.transpose(0, 2, 1)
        recip = (
            np.sum(w_dir * ((Crc - Css) ** 2 + (Crs + Csc) ** 2))
            + np.sum(w_mir * ((Crc + Css) ** 2 + (Csc - Crs) ** 2))
        )
        # the three surviving |n|=10 axis modes, exact on host
        lo = b * N_PER
        qs = q[lo : lo + N_PER, 0].astype(np.float64)
        minv = np.linalg.inv(cell[b].astype(np.float64))
        phi = (r[lo : lo + N_PER].astype(np.float64) @ minv) % 1.0
        for d, widx in ((0, (NK, NK, NK)), (1, (0, 2 * NK, NK)), (2, (0, NK, 2 * NK))):
            ang = TWOPI * NK * phi[:, d]
            sr = np.sum(qs * np.cos(ang)); si = np.sum(qs * np.sin(ang))
            recip += wb[widx] * (sr ** 2 + si ** 2)
        self_e = np.sum(qs ** 2) / (SIGMA * TWOPI ** 1.5)
        pot[b] = (recip - self_e) * NORM
    return pot.astype(np.float32)
